# revision 7
# baseline (speedup 1.0000x reference)
"""GRU-cell-variant kernel for Trainium2, data-parallel over batch on 8 cores.

Reference (per batch row b, hidden size H=1024):
    gates = sigmoid(x @ W_ih + b_ih + h @ W_hh + b_hh)   # [B, 2H]
    z, r  = gates[:, :H], gates[:, H:]
    cand  = tanh(x @ W_c + b_c + r * (h @ W_hc + b_hc))
    out   = (1 - z) * h + z * cand

Design:
  - 8-way batch shard (1024 rows/core), weights replicated. No collectives.
  - Everything on-chip is computed TRANSPOSED: out.T[o, b]. That way weight
    tiles [k, o] load naturally as the stationary operand, host-pre-transposed
    x.T / h.T serve as the moving operand, and all biases are per-partition
    (free bias-add on the ACT engine).
  - Matmuls in fp16 (1 cycle/row on the PE) with fp32 PSUM accumulation;
    elementwise math and h-residual in fp32.
  - Host packs weights/activations into the exact SBUF layouts so every DMA
    is a dense 2D copy with >=2KB per-partition lines.
"""

import numpy as np

import concourse.bass as bass
import concourse.mybir as mybir
import concourse.tile as tile
from concourse import bacc
from concourse.bass_utils import run_bass_kernel_spmd

N_CORES = 8
B = 8192
H = 1024
BL = B // N_CORES  # batch rows per core
P = 128
KC = H // P  # 8 contraction chunks of 128 per 1024-wide operand
NJ = H // P  # 8 hidden-dim tiles
NB = BL // 512  # 2 moving halves of 512 batch columns

F16 = mybir.dt.float16
F32 = mybir.dt.float32
AF = mybir.ActivationFunctionType
ALU = mybir.AluOpType

_CACHE = {}


def _build_program():
    nc = bacc.Bacc(
        "TRN2",
        target_bir_lowering=False,
        debug=False,
        enable_asserts=False,
        num_devices=N_CORES,
    )

    # DRAM inputs, already packed on the host into SBUF-friendly layouts.
    # xT/hT:  [p, kc*BL + b]        = x[b, kc*128 + p]           (fp16)
    # hT32:   same layout, fp32 (residual path)
    # Wg:     [p, t*2048 + kc*128 + jj] = Wg_full[kc*128+p, t*128+jj]  (fp16)
    #          t in [0,16): gate output tile; kc in [0,16): contraction over [x;h]
    # Wc/Whc: [p, j*1024 + kc*128 + jj] = W[kc*128+p, j*128+jj]  (fp16)
    # bg:     [p, t] = (b_ih+b_hh)[t*128+p]; bc/bhc analogous.
    xT = nc.dram_tensor("xT", [P, KC * BL], F16, kind="ExternalInput").ap()
    hT = nc.dram_tensor("hT", [P, KC * BL], F16, kind="ExternalInput").ap()
    hT32 = nc.dram_tensor("hT32", [P, NJ * BL], F32, kind="ExternalInput").ap()
    Wg = nc.dram_tensor("Wg", [P, 16 * 2048], F16, kind="ExternalInput").ap()
    Wc = nc.dram_tensor("Wc", [P, NJ * H], F16, kind="ExternalInput").ap()
    Whc = nc.dram_tensor("Whc", [P, NJ * H], F16, kind="ExternalInput").ap()
    bg = nc.dram_tensor("bg", [P, 16], F32, kind="ExternalInput").ap()
    bc = nc.dram_tensor("bc", [P, NJ], F32, kind="ExternalInput").ap()
    bhc = nc.dram_tensor("bhc", [P, NJ], F32, kind="ExternalInput").ap()
    outT = nc.dram_tensor("outT", [P, NJ * BL], F32, kind="ExternalOutput").ap()

    with tile.TileContext(nc) as tc:
        with (
            tc.tile_pool(name="const", bufs=1) as cpool,
            tc.tile_pool(name="wg", bufs=4) as wgpool,
            tc.tile_pool(name="wsm", bufs=4) as wsmpool,
            tc.tile_pool(name="psum", bufs=8, space="PSUM") as ppool,
            tc.tile_pool(name="gates", bufs=6) as gpool,
            tc.tile_pool(name="work", bufs=10) as wpool,
        ):
            # Small constants first (they gate the ACT ops).
            bg_sb = cpool.tile([P, 16], F32, tag="bg")
            nc.sync.dma_start(bg_sb[:], bg[:])
            bc_sb = cpool.tile([P, NJ], F32, tag="bc")
            nc.sync.dma_start(bc_sb[:], bc[:])
            bhc_sb = cpool.tile([P, NJ], F32, tag="bhc")
            nc.sync.dma_start(bhc_sb[:], bhc[:])

            # Resident activations, loaded in per-kc chunks so the first
            # matmuls only wait on the first 128KB-256KB of traffic instead
            # of the full 8MB input preamble. hT32 (residual path, fp32) is
            # streamed per-j inside the loop — it isn't needed until the
            # first elementwise stage.
            xT_sb = cpool.tile([P, KC * BL], F16, tag="xT")
            hT_sb = cpool.tile([P, KC * BL], F16, tag="hT")
            hT32_sb = cpool.tile([P, NJ * BL], F32, tag="hT32")

            def gate_matmuls(psum, w_sb, b0):
                # accumulate over [x;h]: kc<8 reads xT, kc>=8 reads hT
                for kc in range(2 * KC):
                    src = xT_sb if kc < KC else hT_sb
                    off = (kc % KC) * BL + b0
                    nc.tensor.matmul(
                        psum[:],
                        lhsT=w_sb[:, kc * P : (kc + 1) * P],
                        rhs=src[:, off : off + 512],
                        start=(kc == 0),
                        stop=(kc == 2 * KC - 1),
                    )

            def cand_matmuls(psum, w_sb, src, b0):
                for kc in range(KC):
                    off = kc * BL + b0
                    nc.tensor.matmul(
                        psum[:],
                        lhsT=w_sb[:, kc * P : (kc + 1) * P],
                        rhs=src[:, off : off + 512],
                        start=(kc == 0),
                        stop=(kc == KC - 1),
                    )

            def load_wg(dst, t, chunks=1):
                cw = 2048 // chunks
                for c in range(chunks):
                    nc.sync.dma_start(
                        dst[:, c * cw : (c + 1) * cw],
                        Wg[:, t * 2048 + c * cw : t * 2048 + (c + 1) * cw],
                    )

            def load_act_half(dst, src, b0, kc_start=0):
                # one [128, 512] chunk per kc for batch-half starting at b0
                for kc in range(kc_start, KC):
                    o = kc * BL + b0
                    nc.sync.dma_start(dst[:, o : o + 512], src[:, o : o + 512])

            for j in range(NJ):
                wz = wgpool.tile([P, 2048], F16, tag="wg")
                wr = wgpool.tile([P, 2048], F16, tag="wg")
                whc_w = wsmpool.tile([P, H], F16, tag="wsm")
                wc_w = wsmpool.tile([P, H], F16, tag="wsm")
                if j == 0:
                    # Cold-start feed order: everything the first (z, b=0)
                    # psum group needs — b=0 halves of x and h plus the z
                    # weights — then the rest of j=0, then the b=1 halves.
                    nc.sync.dma_start(xT_sb[:, 0:512], xT[:, 0:512])
                    load_wg(wz, 0, chunks=4)
                    load_act_half(xT_sb, xT, 0, kc_start=1)
                    load_act_half(hT_sb, hT, 0)
                    load_wg(wr, NJ, chunks=4)
                    nc.sync.dma_start(whc_w[:], Whc[:, 0:H])
                    nc.sync.dma_start(wc_w[:], Wc[:, 0:H])
                    load_act_half(xT_sb, xT, 512)
                    load_act_half(hT_sb, hT, 512)
                else:
                    load_wg(wz, j)
                    load_wg(wr, NJ + j)
                    nc.sync.dma_start(whc_w[:], Whc[:, j * H : (j + 1) * H])
                    nc.sync.dma_start(wc_w[:], Wc[:, j * H : (j + 1) * H])
                # residual-path h (fp32) rides the idle GpSimd queue
                nc.gpsimd.dma_start(
                    hT32_sb[:, j * BL : (j + 1) * BL], hT32[:, j * BL : (j + 1) * BL]
                )

                for b in range(NB):
                    b0 = b * 512
                    hoff = j * BL + b0  # slice of hidden tile j in [p, j*BL+b] layout

                    pz = ppool.tile([P, 512], F32, tag="ps")
                    gate_matmuls(pz, wz, b0)
                    z_sb = gpool.tile([P, 512], F32, tag="g")
                    nc.scalar.activation(z_sb[:], pz[:], AF.Sigmoid, bias=bg_sb[:, j : j + 1])
                    # zh = (z - 1) * h, computed off the critical path so the
                    # post-tanh chain is only mul + subtract
                    zh = wpool.tile([P, 512], F32, tag="w")
                    nc.vector.scalar_tensor_tensor(
                        zh[:], z_sb[:], 1.0, hT32_sb[:, hoff : hoff + 512],
                        ALU.subtract, ALU.mult,
                    )

                    pr = ppool.tile([P, 512], F32, tag="ps")
                    gate_matmuls(pr, wr, b0)
                    r_sb = gpool.tile([P, 512], F32, tag="g")
                    nc.scalar.activation(
                        r_sb[:], pr[:], AF.Sigmoid, bias=bg_sb[:, NJ + j : NJ + j + 1]
                    )

                    ph = ppool.tile([P, 512], F32, tag="ps")
                    cand_matmuls(ph, whc_w, hT_sb, b0)
                    px = ppool.tile([P, 512], F32, tag="ps")
                    cand_matmuls(px, wc_w, xT_sb, b0)

                    # rh = (hc + b_hc) * r   (one DVE op)
                    rh = wpool.tile([P, 512], F32, tag="w")
                    nc.vector.scalar_tensor_tensor(
                        rh[:], ph[:], bhc_sb[:, j : j + 1], r_sb[:], ALU.add, ALU.mult
                    )
                    s = wpool.tile([P, 512], F32, tag="w")
                    nc.vector.tensor_add(s[:], px[:], rh[:])
                    cand = wpool.tile([P, 512], F32, tag="w")
                    nc.scalar.activation(cand[:], s[:], AF.Tanh, bias=bc_sb[:, j : j + 1])

                    # out = z*cand - (z-1)*h
                    m = wpool.tile([P, 512], F32, tag="w")
                    nc.vector.tensor_mul(m[:], z_sb[:], cand[:])
                    o_sb = wpool.tile([P, 512], F32, tag="w")
                    nc.vector.tensor_sub(o_sb[:], m[:], zh[:])
                    nc.gpsimd.dma_start(outT[:, hoff : hoff + 512], o_sb[:])

    nc.compile()
    return nc


def _pack_weights(W_ih, b_ih, W_hh, b_hh, W_c, b_c, W_hc, b_hc):
    f16 = np.float16
    Wg_full = np.concatenate([W_ih, W_hh], axis=0)  # [2H, 2H] = [k, o]
    WgH = np.ascontiguousarray(
        Wg_full.reshape(16, P, 16, P).transpose(1, 2, 0, 3).reshape(P, 16 * 2048)
    ).astype(f16)
    WcH = np.ascontiguousarray(
        W_c.reshape(KC, P, NJ, P).transpose(1, 2, 0, 3).reshape(P, NJ * H)
    ).astype(f16)
    WhcH = np.ascontiguousarray(
        W_hc.reshape(KC, P, NJ, P).transpose(1, 2, 0, 3).reshape(P, NJ * H)
    ).astype(f16)
    bgH = np.ascontiguousarray((b_ih + b_hh).reshape(16, P).T).astype(np.float32)
    bcH = np.ascontiguousarray(b_c.reshape(NJ, P).T).astype(np.float32)
    bhcH = np.ascontiguousarray(b_hc.reshape(NJ, P).T).astype(np.float32)
    return WgH, WcH, WhcH, bgH, bcH, bhcH


def _pack_acts(a, dtype):
    # [BL, H] -> [p, kc*BL + b] with a[b, kc*128+p]
    return np.ascontiguousarray(
        a.T.reshape(KC, P, BL).transpose(1, 0, 2).reshape(P, KC * BL)
    ).astype(dtype)


def kernel(input, hx, W_ih, b_ih, W_hh, b_hh, W_c, b_c, W_hc, b_hc):
    input = np.asarray(input, np.float32)
    hx = np.asarray(hx, np.float32)
    if "nc" not in _CACHE:
        _CACHE["nc"] = _build_program()
    nc = _CACHE["nc"]

    WgH, WcH, WhcH, bgH, bcH, bhcH = _pack_weights(
        np.asarray(W_ih, np.float32), np.asarray(b_ih, np.float32),
        np.asarray(W_hh, np.float32), np.asarray(b_hh, np.float32),
        np.asarray(W_c, np.float32), np.asarray(b_c, np.float32),
        np.asarray(W_hc, np.float32), np.asarray(b_hc, np.float32),
    )

    in_maps = []
    for i in range(N_CORES):
        xs = input[i * BL : (i + 1) * BL]
        hs = hx[i * BL : (i + 1) * BL]
        in_maps.append(
            {
                "xT": _pack_acts(xs, np.float16),
                "hT": _pack_acts(hs, np.float16),
                "hT32": _pack_acts(hs, np.float32),
                "Wg": WgH,
                "Wc": WcH,
                "Whc": WhcH,
                "bg": bgH,
                "bc": bcH,
                "bhc": bhcH,
            }
        )

    res = run_bass_kernel_spmd(nc, in_maps, core_ids=list(range(N_CORES)))
    out = np.empty((B, H), np.float32)
    for i, r in enumerate(res.results):
        o = r["outT"].reshape(P, NJ, BL).transpose(2, 1, 0).reshape(BL, H)
        out[i * BL : (i + 1) * BL] = o
    return out


# revision 8
# speedup vs baseline: 1.0330x; 1.0330x over previous
"""GRU-cell-variant kernel for Trainium2, data-parallel over batch on 8 cores.

Reference (per batch row b, hidden size H=1024):
    gates = sigmoid(x @ W_ih + b_ih + h @ W_hh + b_hh)   # [B, 2H]
    z, r  = gates[:, :H], gates[:, H:]
    cand  = tanh(x @ W_c + b_c + r * (h @ W_hc + b_hc))
    out   = (1 - z) * h + z * cand

Design:
  - 8-way batch shard (1024 rows/core), weights replicated. No collectives.
  - Everything on-chip is computed TRANSPOSED: out.T[o, b]. That way weight
    tiles [k, o] load naturally as the stationary operand, host-pre-transposed
    x.T / h.T serve as the moving operand, and all biases are per-partition
    (free bias-add on the ACT engine).
  - Matmuls in fp16 (1 cycle/row on the PE) with fp32 PSUM accumulation;
    elementwise math and h-residual in fp32.
  - Host packs weights/activations into the exact SBUF layouts so every DMA
    is a dense 2D copy with >=2KB per-partition lines.
"""

import numpy as np

import concourse.bass as bass
import concourse.mybir as mybir
import concourse.tile as tile
from concourse import bacc
from concourse.bass_utils import run_bass_kernel_spmd

N_CORES = 8
B = 8192
H = 1024
BL = B // N_CORES  # batch rows per core
P = 128
KC = H // P  # 8 contraction chunks of 128 per 1024-wide operand
NJ = H // P  # 8 hidden-dim tiles
NB = BL // 512  # 2 moving halves of 512 batch columns

F16 = mybir.dt.float16
F32 = mybir.dt.float32
AF = mybir.ActivationFunctionType
ALU = mybir.AluOpType

_CACHE = {}


def _build_program():
    nc = bacc.Bacc(
        "TRN2",
        target_bir_lowering=False,
        debug=False,
        enable_asserts=False,
        num_devices=N_CORES,
    )

    # DRAM inputs, already packed on the host into SBUF-friendly layouts.
    # xT/hT:  [p, kc*BL + b]        = x[b, kc*128 + p]           (fp16)
    # hT32:   same layout, fp32 (residual path)
    # Wg:     [p, t*2048 + kc*128 + jj] = Wg_full[kc*128+p, t*128+jj]  (fp16)
    #          t in [0,16): gate output tile; kc in [0,16): contraction over [x;h]
    # Wc/Whc: [p, j*1024 + kc*128 + jj] = W[kc*128+p, j*128+jj]  (fp16)
    # bg:     [p, t] = (b_ih+b_hh)[t*128+p]; bc/bhc analogous.
    xT = nc.dram_tensor("xT", [P, KC * BL], F16, kind="ExternalInput").ap()
    hT = nc.dram_tensor("hT", [P, KC * BL], F16, kind="ExternalInput").ap()
    hT32 = nc.dram_tensor("hT32", [P, NJ * BL], F32, kind="ExternalInput").ap()
    Wg = nc.dram_tensor("Wg", [P, 16 * 2048], F16, kind="ExternalInput").ap()
    Wc = nc.dram_tensor("Wc", [P, NJ * H], F16, kind="ExternalInput").ap()
    Whc = nc.dram_tensor("Whc", [P, NJ * H], F16, kind="ExternalInput").ap()
    bg = nc.dram_tensor("bg", [P, 16], F32, kind="ExternalInput").ap()
    bc = nc.dram_tensor("bc", [P, NJ], F32, kind="ExternalInput").ap()
    bhc = nc.dram_tensor("bhc", [P, NJ], F32, kind="ExternalInput").ap()
    outT = nc.dram_tensor("outT", [P, NJ * BL], F32, kind="ExternalOutput").ap()

    with tile.TileContext(nc) as tc:
        with (
            tc.tile_pool(name="const", bufs=1) as cpool,
            tc.tile_pool(name="wg", bufs=4) as wgpool,
            tc.tile_pool(name="wsm", bufs=4) as wsmpool,
            tc.tile_pool(name="psum", bufs=8, space="PSUM") as ppool,
            tc.tile_pool(name="gates", bufs=6) as gpool,
            tc.tile_pool(name="work", bufs=10) as wpool,
        ):
            # Small constants first (they gate the ACT ops).
            bg_sb = cpool.tile([P, 16], F32, tag="bg")
            nc.sync.dma_start(bg_sb[:], bg[:])
            bc_sb = cpool.tile([P, NJ], F32, tag="bc")
            nc.sync.dma_start(bc_sb[:], bc[:])
            bhc_sb = cpool.tile([P, NJ], F32, tag="bhc")
            nc.sync.dma_start(bhc_sb[:], bhc[:])

            # Resident activations, loaded in per-kc chunks so the first
            # matmuls only wait on the first 128KB-256KB of traffic instead
            # of the full 8MB input preamble. hT32 (residual path, fp32) is
            # streamed per-j inside the loop — it isn't needed until the
            # first elementwise stage.
            xT_sb = cpool.tile([P, KC * BL], F16, tag="xT")
            hT_sb = cpool.tile([P, KC * BL], F16, tag="hT")
            hT32_sb = cpool.tile([P, NJ * BL], F32, tag="hT32")

            def gate_matmuls(psum, w_sb, b0):
                # accumulate over [x;h]: kc<8 reads xT, kc>=8 reads hT
                for kc in range(2 * KC):
                    src = xT_sb if kc < KC else hT_sb
                    off = (kc % KC) * BL + b0
                    nc.tensor.matmul(
                        psum[:],
                        lhsT=w_sb[:, kc * P : (kc + 1) * P],
                        rhs=src[:, off : off + 512],
                        start=(kc == 0),
                        stop=(kc == 2 * KC - 1),
                    )

            def cand_matmuls(psum, w_sb, src, b0):
                for kc in range(KC):
                    off = kc * BL + b0
                    nc.tensor.matmul(
                        psum[:],
                        lhsT=w_sb[:, kc * P : (kc + 1) * P],
                        rhs=src[:, off : off + 512],
                        start=(kc == 0),
                        stop=(kc == KC - 1),
                    )

            def load_wg(dst, t, chunks=1):
                cw = 2048 // chunks
                for c in range(chunks):
                    nc.sync.dma_start(
                        dst[:, c * cw : (c + 1) * cw],
                        Wg[:, t * 2048 + c * cw : t * 2048 + (c + 1) * cw],
                    )

            def load_act_half(dst, src, b0, kc_start=0):
                # one [128, 512] chunk per kc for batch-half starting at b0
                for kc in range(kc_start, KC):
                    o = kc * BL + b0
                    nc.sync.dma_start(dst[:, o : o + 512], src[:, o : o + 512])

            for j in range(NJ):
                wz = wgpool.tile([P, 2048], F16, tag="wg")
                wr = wgpool.tile([P, 2048], F16, tag="wg")
                whc_w = wsmpool.tile([P, H], F16, tag="wsm")
                wc_w = wsmpool.tile([P, H], F16, tag="wsm")
                if j == 0:
                    # Cold-start feed order: everything the first (z, b=0)
                    # psum group needs — b=0 halves of x and h plus the z
                    # weights — then the rest of j=0, then the b=1 halves.
                    nc.sync.dma_start(xT_sb[:, 0:512], xT[:, 0:512])
                    load_wg(wz, 0, chunks=4)
                    load_act_half(xT_sb, xT, 0, kc_start=1)
                    load_act_half(hT_sb, hT, 0)
                    load_wg(wr, NJ, chunks=4)
                    nc.sync.dma_start(whc_w[:], Whc[:, 0:H])
                    nc.sync.dma_start(wc_w[:], Wc[:, 0:H])
                    load_act_half(xT_sb, xT, 512)
                    load_act_half(hT_sb, hT, 512)
                else:
                    load_wg(wz, j)
                    load_wg(wr, NJ + j)
                    nc.sync.dma_start(whc_w[:], Whc[:, j * H : (j + 1) * H])
                    nc.sync.dma_start(wc_w[:], Wc[:, j * H : (j + 1) * H])
                # residual-path h (fp32) rides the second HWDGE ring (ACT)
                nc.scalar.dma_start(
                    hT32_sb[:, j * BL : (j + 1) * BL], hT32[:, j * BL : (j + 1) * BL]
                )

                for b in range(NB):
                    b0 = b * 512
                    hoff = j * BL + b0  # slice of hidden tile j in [p, j*BL+b] layout

                    pz = ppool.tile([P, 512], F32, tag="ps")
                    gate_matmuls(pz, wz, b0)
                    z_sb = gpool.tile([P, 512], F32, tag="g")
                    nc.scalar.activation(z_sb[:], pz[:], AF.Sigmoid, bias=bg_sb[:, j : j + 1])
                    # zh = (z - 1) * h, computed off the critical path so the
                    # post-tanh chain is only mul + subtract
                    zh = wpool.tile([P, 512], F32, tag="w")
                    nc.vector.scalar_tensor_tensor(
                        zh[:], z_sb[:], 1.0, hT32_sb[:, hoff : hoff + 512],
                        ALU.subtract, ALU.mult,
                    )

                    pr = ppool.tile([P, 512], F32, tag="ps")
                    gate_matmuls(pr, wr, b0)
                    r_sb = gpool.tile([P, 512], F32, tag="g")
                    nc.scalar.activation(
                        r_sb[:], pr[:], AF.Sigmoid, bias=bg_sb[:, NJ + j : NJ + j + 1]
                    )

                    ph = ppool.tile([P, 512], F32, tag="ps")
                    cand_matmuls(ph, whc_w, hT_sb, b0)
                    px = ppool.tile([P, 512], F32, tag="ps")
                    cand_matmuls(px, wc_w, xT_sb, b0)

                    # rh = (hc + b_hc) * r   (one DVE op)
                    rh = wpool.tile([P, 512], F32, tag="w")
                    nc.vector.scalar_tensor_tensor(
                        rh[:], ph[:], bhc_sb[:, j : j + 1], r_sb[:], ALU.add, ALU.mult
                    )
                    s = wpool.tile([P, 512], F32, tag="w")
                    nc.vector.tensor_add(s[:], px[:], rh[:])
                    cand = wpool.tile([P, 512], F32, tag="w")
                    nc.scalar.activation(cand[:], s[:], AF.Tanh, bias=bc_sb[:, j : j + 1])

                    # out = z*cand - (z-1)*h
                    m = wpool.tile([P, 512], F32, tag="w")
                    nc.vector.tensor_mul(m[:], z_sb[:], cand[:])
                    o_sb = wpool.tile([P, 512], F32, tag="w")
                    nc.vector.tensor_sub(o_sb[:], m[:], zh[:])
                    nc.scalar.dma_start(outT[:, hoff : hoff + 512], o_sb[:])

    nc.compile()
    return nc


def _pack_weights(W_ih, b_ih, W_hh, b_hh, W_c, b_c, W_hc, b_hc):
    f16 = np.float16
    Wg_full = np.concatenate([W_ih, W_hh], axis=0)  # [2H, 2H] = [k, o]
    WgH = np.ascontiguousarray(
        Wg_full.reshape(16, P, 16, P).transpose(1, 2, 0, 3).reshape(P, 16 * 2048)
    ).astype(f16)
    WcH = np.ascontiguousarray(
        W_c.reshape(KC, P, NJ, P).transpose(1, 2, 0, 3).reshape(P, NJ * H)
    ).astype(f16)
    WhcH = np.ascontiguousarray(
        W_hc.reshape(KC, P, NJ, P).transpose(1, 2, 0, 3).reshape(P, NJ * H)
    ).astype(f16)
    bgH = np.ascontiguousarray((b_ih + b_hh).reshape(16, P).T).astype(np.float32)
    bcH = np.ascontiguousarray(b_c.reshape(NJ, P).T).astype(np.float32)
    bhcH = np.ascontiguousarray(b_hc.reshape(NJ, P).T).astype(np.float32)
    return WgH, WcH, WhcH, bgH, bcH, bhcH


def _pack_acts(a, dtype):
    # [BL, H] -> [p, kc*BL + b] with a[b, kc*128+p]
    return np.ascontiguousarray(
        a.T.reshape(KC, P, BL).transpose(1, 0, 2).reshape(P, KC * BL)
    ).astype(dtype)


def kernel(input, hx, W_ih, b_ih, W_hh, b_hh, W_c, b_c, W_hc, b_hc):
    input = np.asarray(input, np.float32)
    hx = np.asarray(hx, np.float32)
    if "nc" not in _CACHE:
        _CACHE["nc"] = _build_program()
    nc = _CACHE["nc"]

    WgH, WcH, WhcH, bgH, bcH, bhcH = _pack_weights(
        np.asarray(W_ih, np.float32), np.asarray(b_ih, np.float32),
        np.asarray(W_hh, np.float32), np.asarray(b_hh, np.float32),
        np.asarray(W_c, np.float32), np.asarray(b_c, np.float32),
        np.asarray(W_hc, np.float32), np.asarray(b_hc, np.float32),
    )

    in_maps = []
    for i in range(N_CORES):
        xs = input[i * BL : (i + 1) * BL]
        hs = hx[i * BL : (i + 1) * BL]
        in_maps.append(
            {
                "xT": _pack_acts(xs, np.float16),
                "hT": _pack_acts(hs, np.float16),
                "hT32": _pack_acts(hs, np.float32),
                "Wg": WgH,
                "Wc": WcH,
                "Whc": WhcH,
                "bg": bgH,
                "bc": bcH,
                "bhc": bhcH,
            }
        )

    res = run_bass_kernel_spmd(nc, in_maps, core_ids=list(range(N_CORES)))
    out = np.empty((B, H), np.float32)
    for i, r in enumerate(res.results):
        o = r["outT"].reshape(P, NJ, BL).transpose(2, 1, 0).reshape(BL, H)
        out[i * BL : (i + 1) * BL] = o
    return out


# revision 10
# speedup vs baseline: 1.0350x; 1.0020x over previous
"""GRU-cell-variant kernel for Trainium2, data-parallel over batch on 8 cores.

Reference (per batch row b, hidden size H=1024):
    gates = sigmoid(x @ W_ih + b_ih + h @ W_hh + b_hh)   # [B, 2H]
    z, r  = gates[:, :H], gates[:, H:]
    cand  = tanh(x @ W_c + b_c + r * (h @ W_hc + b_hc))
    out   = (1 - z) * h + z * cand

Design:
  - 8-way batch shard (1024 rows/core), weights replicated. No collectives.
  - Everything on-chip is computed TRANSPOSED: out.T[o, b]. That way weight
    tiles [k, o] load naturally as the stationary operand, host-pre-transposed
    x.T / h.T serve as the moving operand, and all biases are per-partition
    (free bias-add on the ACT engine).
  - Matmuls in fp16 (1 cycle/row on the PE) with fp32 PSUM accumulation;
    elementwise math and h-residual in fp32.
  - Host packs weights/activations into the exact SBUF layouts so every DMA
    is a dense 2D copy with >=2KB per-partition lines.
"""

import numpy as np

import concourse.bass as bass
import concourse.mybir as mybir
import concourse.tile as tile
from concourse import bacc
from concourse.bass_utils import run_bass_kernel_spmd

N_CORES = 8
B = 8192
H = 1024
BL = B // N_CORES  # batch rows per core
P = 128
KC = H // P  # 8 contraction chunks of 128 per 1024-wide operand
NJ = H // P  # 8 hidden-dim tiles
NB = BL // 512  # 2 moving halves of 512 batch columns

F16 = mybir.dt.float16
F32 = mybir.dt.float32
AF = mybir.ActivationFunctionType
ALU = mybir.AluOpType

_CACHE = {}


def _build_program():
    nc = bacc.Bacc(
        "TRN2",
        target_bir_lowering=False,
        debug=False,
        enable_asserts=False,
        num_devices=N_CORES,
    )

    # DRAM inputs, already packed on the host into SBUF-friendly layouts.
    # xT/hT:  [p, kc*BL + b]        = x[b, kc*128 + p]           (fp16)
    # hT32:   same layout, fp32 (residual path)
    # Wg:     [p, t*2048 + kc*128 + jj] = Wg_full[kc*128+p, t*128+jj]  (fp16)
    #          t in [0,16): gate output tile; kc in [0,16): contraction over [x;h]
    # Wc/Whc: [p, j*1024 + kc*128 + jj] = W[kc*128+p, j*128+jj]  (fp16)
    # bg:     [p, t] = (b_ih+b_hh)[t*128+p]; bc/bhc analogous.
    xT = nc.dram_tensor("xT", [P, KC * BL], F16, kind="ExternalInput").ap()
    hT = nc.dram_tensor("hT", [P, KC * BL], F16, kind="ExternalInput").ap()
    hT32 = nc.dram_tensor("hT32", [P, NJ * BL], F32, kind="ExternalInput").ap()
    Wg = nc.dram_tensor("Wg", [P, 16 * 2048], F16, kind="ExternalInput").ap()
    Wc = nc.dram_tensor("Wc", [P, NJ * H], F16, kind="ExternalInput").ap()
    Whc = nc.dram_tensor("Whc", [P, NJ * H], F16, kind="ExternalInput").ap()
    bg = nc.dram_tensor("bg", [P, 16], F32, kind="ExternalInput").ap()
    bc = nc.dram_tensor("bc", [P, NJ], F32, kind="ExternalInput").ap()
    bhc = nc.dram_tensor("bhc", [P, NJ], F32, kind="ExternalInput").ap()
    outT = nc.dram_tensor("outT", [P, NJ * BL], F32, kind="ExternalOutput").ap()

    with tile.TileContext(nc) as tc:
        with (
            tc.tile_pool(name="const", bufs=1) as cpool,
            tc.tile_pool(name="wg", bufs=4) as wgpool,
            tc.tile_pool(name="wsm", bufs=4) as wsmpool,
            tc.tile_pool(name="psum", bufs=8, space="PSUM") as ppool,
            tc.tile_pool(name="gates", bufs=6) as gpool,
            tc.tile_pool(name="work", bufs=10) as wpool,
        ):
            # Small constants ride the ACT HWDGE ring: each DMA issue costs
            # ~600ns of sequencer time, and the sync ring's early issue slots
            # are the critical resource for feeding the first matmuls.
            bg_sb = cpool.tile([P, 16], F32, tag="bg")
            nc.scalar.dma_start(bg_sb[:], bg[:])
            bc_sb = cpool.tile([P, NJ], F32, tag="bc")
            nc.scalar.dma_start(bc_sb[:], bc[:])
            bhc_sb = cpool.tile([P, NJ], F32, tag="bhc")
            nc.scalar.dma_start(bhc_sb[:], bhc[:])

            # Resident activations, loaded in per-kc chunks so the first
            # matmuls only wait on the first 128KB-256KB of traffic instead
            # of the full 8MB input preamble. hT32 (residual path, fp32) is
            # streamed per-j inside the loop — it isn't needed until the
            # first elementwise stage.
            xT_sb = cpool.tile([P, KC * BL], F16, tag="xT")
            hT_sb = cpool.tile([P, KC * BL], F16, tag="hT")
            hT32_sb = cpool.tile([P, NJ * BL], F32, tag="hT32")

            def gate_matmuls(psum, w_sb, b0):
                # accumulate over [x;h]: kc<8 reads xT, kc>=8 reads hT
                for kc in range(2 * KC):
                    src = xT_sb if kc < KC else hT_sb
                    off = (kc % KC) * BL + b0
                    nc.tensor.matmul(
                        psum[:],
                        lhsT=w_sb[:, kc * P : (kc + 1) * P],
                        rhs=src[:, off : off + 512],
                        start=(kc == 0),
                        stop=(kc == 2 * KC - 1),
                    )

            def cand_matmuls(psum, w_sb, src, b0):
                for kc in range(KC):
                    off = kc * BL + b0
                    nc.tensor.matmul(
                        psum[:],
                        lhsT=w_sb[:, kc * P : (kc + 1) * P],
                        rhs=src[:, off : off + 512],
                        start=(kc == 0),
                        stop=(kc == KC - 1),
                    )

            def load_wg(dst, t, chunks=1):
                cw = 2048 // chunks
                for c in range(chunks):
                    nc.sync.dma_start(
                        dst[:, c * cw : (c + 1) * cw],
                        Wg[:, t * 2048 + c * cw : t * 2048 + (c + 1) * cw],
                    )

            # 3D views for merged strided chunk loads: [p, kc, b]
            xs3 = xT_sb[:].rearrange("p (kc b) -> p kc b", kc=KC)
            xd3 = xT.rearrange("p (kc b) -> p kc b", kc=KC)
            hs3 = hT_sb[:].rearrange("p (kc b) -> p kc b", kc=KC)
            hd3 = hT.rearrange("p (kc b) -> p kc b", kc=KC)

            def load_act(dst3, src3, kc0, kc1, b0, bw):
                nc.sync.dma_start(
                    dst3[:, kc0:kc1, b0 : b0 + bw], src3[:, kc0:kc1, b0 : b0 + bw]
                )

            for j in range(NJ):
                wz = wgpool.tile([P, 2048], F16, tag="wg")
                wr = wgpool.tile([P, 2048], F16, tag="wg")
                whc_w = wsmpool.tile([P, H], F16, tag="wsm")
                wc_w = wsmpool.tile([P, H], F16, tag="wsm")
                if j == 0:
                    # Cold-start feed: the sync sequencer issues one DMA per
                    # ~600ns, so the first (z, b=0) group's data is ordered
                    # kc-by-kc — x chunk, matching z-weight chunk, ... —
                    # and everything else is merged into few big transfers.
                    load_act(xs3, xd3, 0, 1, 0, 512)          # x kc0 b0
                    load_wg(wz, 0, chunks=4)                   # c0 covers kc0-3
                    load_act(xs3, xd3, 1, 4, 0, 512)          # x kc1-3 b0
                    load_act(xs3, xd3, 4, 8, 0, 512)          # x kc4-7 b0
                    load_act(hs3, hd3, 0, 4, 0, 512)          # h kc0-3 b0
                    load_act(hs3, hd3, 4, 8, 0, 512)          # h kc4-7 b0
                    load_wg(wr, NJ)                            # one 512KB issue
                    nc.sync.dma_start(whc_w[:], Whc[:, 0:H])
                    nc.sync.dma_start(wc_w[:], Wc[:, 0:H])
                    load_act(xs3, xd3, 0, 8, 512, 512)        # x b1, one issue
                    load_act(hs3, hd3, 0, 8, 512, 512)        # h b1, one issue
                else:
                    load_wg(wz, j)
                    load_wg(wr, NJ + j)
                    nc.sync.dma_start(whc_w[:], Whc[:, j * H : (j + 1) * H])
                    nc.sync.dma_start(wc_w[:], Wc[:, j * H : (j + 1) * H])
                # residual-path h (fp32) rides the second HWDGE ring (ACT)
                nc.scalar.dma_start(
                    hT32_sb[:, j * BL : (j + 1) * BL], hT32[:, j * BL : (j + 1) * BL]
                )

                for b in range(NB):
                    b0 = b * 512
                    hoff = j * BL + b0  # slice of hidden tile j in [p, j*BL+b] layout

                    pz = ppool.tile([P, 512], F32, tag="ps")
                    gate_matmuls(pz, wz, b0)
                    z_sb = gpool.tile([P, 512], F32, tag="g")
                    nc.scalar.activation(z_sb[:], pz[:], AF.Sigmoid, bias=bg_sb[:, j : j + 1])
                    # zh = (z - 1) * h, computed off the critical path so the
                    # post-tanh chain is only mul + subtract
                    zh = wpool.tile([P, 512], F32, tag="w")
                    nc.vector.scalar_tensor_tensor(
                        zh[:], z_sb[:], 1.0, hT32_sb[:, hoff : hoff + 512],
                        ALU.subtract, ALU.mult,
                    )

                    pr = ppool.tile([P, 512], F32, tag="ps")
                    gate_matmuls(pr, wr, b0)
                    r_sb = gpool.tile([P, 512], F32, tag="g")
                    nc.scalar.activation(
                        r_sb[:], pr[:], AF.Sigmoid, bias=bg_sb[:, NJ + j : NJ + j + 1]
                    )

                    ph = ppool.tile([P, 512], F32, tag="ps")
                    cand_matmuls(ph, whc_w, hT_sb, b0)
                    px = ppool.tile([P, 512], F32, tag="ps")
                    cand_matmuls(px, wc_w, xT_sb, b0)

                    # rh = (hc + b_hc) * r   (one DVE op)
                    rh = wpool.tile([P, 512], F32, tag="w")
                    nc.vector.scalar_tensor_tensor(
                        rh[:], ph[:], bhc_sb[:, j : j + 1], r_sb[:], ALU.add, ALU.mult
                    )
                    s = wpool.tile([P, 512], F32, tag="w")
                    nc.vector.tensor_add(s[:], px[:], rh[:])
                    cand = wpool.tile([P, 512], F32, tag="w")
                    nc.scalar.activation(cand[:], s[:], AF.Tanh, bias=bc_sb[:, j : j + 1])

                    # out = z*cand - (z-1)*h
                    m = wpool.tile([P, 512], F32, tag="w")
                    nc.vector.tensor_mul(m[:], z_sb[:], cand[:])
                    o_sb = wpool.tile([P, 512], F32, tag="w")
                    nc.vector.tensor_sub(o_sb[:], m[:], zh[:])
                    nc.scalar.dma_start(outT[:, hoff : hoff + 512], o_sb[:])

    nc.compile()
    return nc


def _pack_weights(W_ih, b_ih, W_hh, b_hh, W_c, b_c, W_hc, b_hc):
    f16 = np.float16
    Wg_full = np.concatenate([W_ih, W_hh], axis=0)  # [2H, 2H] = [k, o]
    WgH = np.ascontiguousarray(
        Wg_full.reshape(16, P, 16, P).transpose(1, 2, 0, 3).reshape(P, 16 * 2048)
    ).astype(f16)
    WcH = np.ascontiguousarray(
        W_c.reshape(KC, P, NJ, P).transpose(1, 2, 0, 3).reshape(P, NJ * H)
    ).astype(f16)
    WhcH = np.ascontiguousarray(
        W_hc.reshape(KC, P, NJ, P).transpose(1, 2, 0, 3).reshape(P, NJ * H)
    ).astype(f16)
    bgH = np.ascontiguousarray((b_ih + b_hh).reshape(16, P).T).astype(np.float32)
    bcH = np.ascontiguousarray(b_c.reshape(NJ, P).T).astype(np.float32)
    bhcH = np.ascontiguousarray(b_hc.reshape(NJ, P).T).astype(np.float32)
    return WgH, WcH, WhcH, bgH, bcH, bhcH


def _pack_acts(a, dtype):
    # [BL, H] -> [p, kc*BL + b] with a[b, kc*128+p]
    return np.ascontiguousarray(
        a.T.reshape(KC, P, BL).transpose(1, 0, 2).reshape(P, KC * BL)
    ).astype(dtype)


def kernel(input, hx, W_ih, b_ih, W_hh, b_hh, W_c, b_c, W_hc, b_hc):
    input = np.asarray(input, np.float32)
    hx = np.asarray(hx, np.float32)
    if "nc" not in _CACHE:
        _CACHE["nc"] = _build_program()
    nc = _CACHE["nc"]

    WgH, WcH, WhcH, bgH, bcH, bhcH = _pack_weights(
        np.asarray(W_ih, np.float32), np.asarray(b_ih, np.float32),
        np.asarray(W_hh, np.float32), np.asarray(b_hh, np.float32),
        np.asarray(W_c, np.float32), np.asarray(b_c, np.float32),
        np.asarray(W_hc, np.float32), np.asarray(b_hc, np.float32),
    )

    in_maps = []
    for i in range(N_CORES):
        xs = input[i * BL : (i + 1) * BL]
        hs = hx[i * BL : (i + 1) * BL]
        in_maps.append(
            {
                "xT": _pack_acts(xs, np.float16),
                "hT": _pack_acts(hs, np.float16),
                "hT32": _pack_acts(hs, np.float32),
                "Wg": WgH,
                "Wc": WcH,
                "Whc": WhcH,
                "bg": bgH,
                "bc": bcH,
                "bhc": bhcH,
            }
        )

    res = run_bass_kernel_spmd(nc, in_maps, core_ids=list(range(N_CORES)))
    out = np.empty((B, H), np.float32)
    for i, r in enumerate(res.results):
        o = r["outT"].reshape(P, NJ, BL).transpose(2, 1, 0).reshape(BL, H)
        out[i * BL : (i + 1) * BL] = o
    return out


# revision 13
# speedup vs baseline: 1.0469x; 1.0115x over previous
"""GRU-cell-variant kernel for Trainium2, data-parallel over batch on 8 cores.

Reference (per batch row b, hidden size H=1024):
    gates = sigmoid(x @ W_ih + b_ih + h @ W_hh + b_hh)   # [B, 2H]
    z, r  = gates[:, :H], gates[:, H:]
    cand  = tanh(x @ W_c + b_c + r * (h @ W_hc + b_hc))
    out   = (1 - z) * h + z * cand

Design:
  - 8-way batch shard (1024 rows/core), weights replicated. No collectives.
  - Everything on-chip is computed TRANSPOSED: out.T[o, b]. That way weight
    tiles [k, o] load naturally as the stationary operand, host-pre-transposed
    x.T / h.T serve as the moving operand, and all biases are per-partition
    (free bias-add on the ACT engine).
  - Matmuls in fp16 (1 cycle/row on the PE) with fp32 PSUM accumulation;
    elementwise math and h-residual in fp32.
  - Host packs weights/activations into the exact SBUF layouts so every DMA
    is a dense 2D copy with >=2KB per-partition lines.
"""

import numpy as np

import concourse.bass as bass
import concourse.mybir as mybir
import concourse.tile as tile
from concourse import bacc
from concourse.bass_utils import run_bass_kernel_spmd

N_CORES = 8
B = 8192
H = 1024
BL = B // N_CORES  # batch rows per core
P = 128
KC = H // P  # 8 contraction chunks of 128 per 1024-wide operand
NJ = H // P  # 8 hidden-dim tiles
NB = BL // 512  # 2 moving halves of 512 batch columns

F16 = mybir.dt.float16
F32 = mybir.dt.float32
AF = mybir.ActivationFunctionType
ALU = mybir.AluOpType

_CACHE = {}


def _build_program():
    nc = bacc.Bacc(
        "TRN2",
        target_bir_lowering=False,
        debug=False,
        enable_asserts=False,
        num_devices=N_CORES,
    )

    # DRAM inputs, already packed on the host into SBUF-friendly layouts.
    # xT/hT:  [p, kc*BL + b]        = x[b, kc*128 + p]           (fp16)
    # hT32:   same layout, fp32 (residual path)
    # Wg:     [p, t*2048 + kc*128 + jj] = Wg_full[kc*128+p, t*128+jj]  (fp16)
    #          t in [0,16): gate output tile; kc in [0,16): contraction over [x;h]
    # Wc/Whc: [p, j*1024 + kc*128 + jj] = W[kc*128+p, j*128+jj]  (fp16)
    # bg:     [p, t] = (b_ih+b_hh)[t*128+p]; bc/bhc analogous.
    xT = nc.dram_tensor("xT", [P, KC * BL], F16, kind="ExternalInput").ap()
    hT = nc.dram_tensor("hT", [P, KC * BL], F16, kind="ExternalInput").ap()
    hT32 = nc.dram_tensor("hT32", [P, NJ * BL], F32, kind="ExternalInput").ap()
    Wg = nc.dram_tensor("Wg", [P, 16 * 2048], F16, kind="ExternalInput").ap()
    Wc = nc.dram_tensor("Wc", [P, NJ * H], F16, kind="ExternalInput").ap()
    Whc = nc.dram_tensor("Whc", [P, NJ * H], F16, kind="ExternalInput").ap()
    bg = nc.dram_tensor("bg", [P, 16], F32, kind="ExternalInput").ap()
    bc = nc.dram_tensor("bc", [P, NJ], F32, kind="ExternalInput").ap()
    bhc = nc.dram_tensor("bhc", [P, NJ], F32, kind="ExternalInput").ap()
    outT = nc.dram_tensor("outT", [P, NJ * BL], F32, kind="ExternalOutput").ap()

    with tile.TileContext(nc) as tc:
        with (
            tc.tile_pool(name="const", bufs=1) as cpool,
            tc.tile_pool(name="wg", bufs=4) as wgpool,
            tc.tile_pool(name="wsm", bufs=4) as wsmpool,
            tc.tile_pool(name="psum", bufs=8, space="PSUM") as ppool,
            tc.tile_pool(name="gates", bufs=6) as gpool,
            tc.tile_pool(name="work", bufs=10) as wpool,
        ):
            # Small constants ride the ACT HWDGE ring: each DMA issue costs
            # ~600ns of sequencer time, and the sync ring's early issue slots
            # are the critical resource for feeding the first matmuls.
            bg_sb = cpool.tile([P, 16], F32, tag="bg")
            nc.scalar.dma_start(bg_sb[:], bg[:])
            bc_sb = cpool.tile([P, NJ], F32, tag="bc")
            nc.scalar.dma_start(bc_sb[:], bc[:])
            bhc_sb = cpool.tile([P, NJ], F32, tag="bhc")
            nc.scalar.dma_start(bhc_sb[:], bhc[:])

            # Resident activations, loaded in per-kc chunks so the first
            # matmuls only wait on the first 128KB-256KB of traffic instead
            # of the full 8MB input preamble. hT32 (residual path, fp32) is
            # streamed per-j inside the loop — it isn't needed until the
            # first elementwise stage.
            xT_sb = cpool.tile([P, KC * BL], F16, tag="xT")
            hT_sb = cpool.tile([P, KC * BL], F16, tag="hT")
            hT32_sb = cpool.tile([P, NJ * BL], F32, tag="hT32")

            def gate_matmuls(psum, w_sb, b0):
                # accumulate over [x;h]: kc<8 reads xT, kc>=8 reads hT
                for kc in range(2 * KC):
                    src = xT_sb if kc < KC else hT_sb
                    off = (kc % KC) * BL + b0
                    nc.tensor.matmul(
                        psum[:],
                        lhsT=w_sb[:, kc * P : (kc + 1) * P],
                        rhs=src[:, off : off + 512],
                        start=(kc == 0),
                        stop=(kc == 2 * KC - 1),
                    )

            def cand_matmuls(psum, w_sb, src, b0):
                for kc in range(KC):
                    off = kc * BL + b0
                    nc.tensor.matmul(
                        psum[:],
                        lhsT=w_sb[:, kc * P : (kc + 1) * P],
                        rhs=src[:, off : off + 512],
                        start=(kc == 0),
                        stop=(kc == KC - 1),
                    )

            def load_wg(dst, t, chunks=1):
                cw = 2048 // chunks
                for c in range(chunks):
                    nc.sync.dma_start(
                        dst[:, c * cw : (c + 1) * cw],
                        Wg[:, t * 2048 + c * cw : t * 2048 + (c + 1) * cw],
                    )

            # 3D views for merged strided chunk loads: [p, kc, b]
            xs3 = xT_sb[:].rearrange("p (kc b) -> p kc b", kc=KC)
            xd3 = xT.rearrange("p (kc b) -> p kc b", kc=KC)
            hs3 = hT_sb[:].rearrange("p (kc b) -> p kc b", kc=KC)
            hd3 = hT.rearrange("p (kc b) -> p kc b", kc=KC)

            def load_act(dst3, src3, kc0, kc1, b0, bw):
                nc.sync.dma_start(
                    dst3[:, kc0:kc1, b0 : b0 + bw], src3[:, kc0:kc1, b0 : b0 + bw]
                )

            for j in range(NJ):
                wz = wgpool.tile([P, 2048], F16, tag="wg")
                wr = wgpool.tile([P, 2048], F16, tag="wg")
                whc_w = wsmpool.tile([P, H], F16, tag="wsm")
                wc_w = wsmpool.tile([P, H], F16, tag="wsm")
                if j == 0:
                    # Cold-start feed. Two serial resources gate the head:
                    # the sync sequencer (~600ns per DMA issue) and HBM BW
                    # (~344GB/s). Feed in 4-kc bundles that pair activation
                    # chunks with BOTH z and r weight chunks — the r matmuls
                    # reuse the same activations, so each early byte unlocks
                    # twice the PE work and the cold-window deficit stays
                    # within ~2us of ideal.
                    def wgc(dst, t, c):  # [128, 512] chunk c of gate col t
                        nc.sync.dma_start(
                            dst[:, c * 512 : (c + 1) * 512],
                            Wg[:, t * 2048 + c * 512 : t * 2048 + (c + 1) * 512],
                        )

                    load_act(xs3, xd3, 0, 4, 0, 512)   # x kc0-3 b0
                    wgc(wz, 0, 0)
                    wgc(wr, NJ, 0)
                    load_act(xs3, xd3, 4, 8, 0, 512)   # x kc4-7 b0
                    wgc(wz, 0, 1)
                    wgc(wr, NJ, 1)
                    load_act(hs3, hd3, 0, 4, 0, 512)   # h kc0-3 b0
                    wgc(wz, 0, 2)
                    wgc(wr, NJ, 2)
                    load_act(hs3, hd3, 4, 8, 0, 512)   # h kc4-7 b0
                    wgc(wz, 0, 3)
                    wgc(wr, NJ, 3)
                    nc.sync.dma_start(whc_w[:], Whc[:, 0:H])
                    nc.sync.dma_start(wc_w[:], Wc[:, 0:H])
                    load_act(xs3, xd3, 0, 8, 512, 512)  # x b1, one issue
                    load_act(hs3, hd3, 0, 8, 512, 512)  # h b1, one issue
                else:
                    load_wg(wz, j)
                    load_wg(wr, NJ + j)
                    nc.sync.dma_start(whc_w[:], Whc[:, j * H : (j + 1) * H])
                    nc.sync.dma_start(wc_w[:], Wc[:, j * H : (j + 1) * H])
                # residual-path h (fp32) rides the second HWDGE ring (ACT)
                nc.scalar.dma_start(
                    hT32_sb[:, j * BL : (j + 1) * BL], hT32[:, j * BL : (j + 1) * BL]
                )

                for b in range(NB):
                    b0 = b * 512
                    hoff = j * BL + b0  # slice of hidden tile j in [p, j*BL+b] layout

                    pz = ppool.tile([P, 512], F32, tag="ps")
                    if j == 0 and b == 0:
                        # cold start: interleave z/r accumulation in 4-kc
                        # blocks matching the DMA bundle arrival order (PE
                        # executes its stream in order)
                        pr = ppool.tile([P, 512], F32, tag="ps")
                        for c in range(4):
                            for grp, w_sb in ((pz, wz), (pr, wr)):
                                for kc in range(4 * c, 4 * c + 4):
                                    src = xT_sb if kc < KC else hT_sb
                                    off = (kc % KC) * BL + b0
                                    nc.tensor.matmul(
                                        grp[:],
                                        lhsT=w_sb[:, kc * P : (kc + 1) * P],
                                        rhs=src[:, off : off + 512],
                                        start=(kc == 0),
                                        stop=(kc == 2 * KC - 1),
                                    )
                    else:
                        gate_matmuls(pz, wz, b0)
                        pr = None
                    z_sb = gpool.tile([P, 512], F32, tag="g")
                    nc.scalar.activation(z_sb[:], pz[:], AF.Sigmoid, bias=bg_sb[:, j : j + 1])
                    # zh = (z - 1) * h, computed off the critical path so the
                    # post-tanh chain is only mul + subtract
                    zh = wpool.tile([P, 512], F32, tag="w")
                    nc.vector.scalar_tensor_tensor(
                        zh[:], z_sb[:], 1.0, hT32_sb[:, hoff : hoff + 512],
                        ALU.subtract, ALU.mult,
                    )

                    if pr is None:
                        pr = ppool.tile([P, 512], F32, tag="ps")
                        gate_matmuls(pr, wr, b0)
                    r_sb = gpool.tile([P, 512], F32, tag="g")
                    nc.scalar.activation(
                        r_sb[:], pr[:], AF.Sigmoid, bias=bg_sb[:, NJ + j : NJ + j + 1]
                    )

                    ph = ppool.tile([P, 512], F32, tag="ps")
                    cand_matmuls(ph, whc_w, hT_sb, b0)
                    px = ppool.tile([P, 512], F32, tag="ps")
                    cand_matmuls(px, wc_w, xT_sb, b0)

                    # rh = (hc + b_hc) * r   (one DVE op)
                    rh = wpool.tile([P, 512], F32, tag="w")
                    nc.vector.scalar_tensor_tensor(
                        rh[:], ph[:], bhc_sb[:, j : j + 1], r_sb[:], ALU.add, ALU.mult
                    )
                    s = wpool.tile([P, 512], F32, tag="w")
                    nc.vector.tensor_add(s[:], px[:], rh[:])
                    cand = wpool.tile([P, 512], F32, tag="w")
                    nc.scalar.activation(cand[:], s[:], AF.Tanh, bias=bc_sb[:, j : j + 1])

                    # out = z*cand - (z-1)*h
                    m = wpool.tile([P, 512], F32, tag="w")
                    nc.vector.tensor_mul(m[:], z_sb[:], cand[:])
                    o_sb = wpool.tile([P, 512], F32, tag="w")
                    nc.vector.tensor_sub(o_sb[:], m[:], zh[:])
                    nc.scalar.dma_start(outT[:, hoff : hoff + 512], o_sb[:])

    nc.compile()
    return nc


def _pack_weights(W_ih, b_ih, W_hh, b_hh, W_c, b_c, W_hc, b_hc):
    f16 = np.float16
    Wg_full = np.concatenate([W_ih, W_hh], axis=0)  # [2H, 2H] = [k, o]
    WgH = np.ascontiguousarray(
        Wg_full.reshape(16, P, 16, P).transpose(1, 2, 0, 3).reshape(P, 16 * 2048)
    ).astype(f16)
    WcH = np.ascontiguousarray(
        W_c.reshape(KC, P, NJ, P).transpose(1, 2, 0, 3).reshape(P, NJ * H)
    ).astype(f16)
    WhcH = np.ascontiguousarray(
        W_hc.reshape(KC, P, NJ, P).transpose(1, 2, 0, 3).reshape(P, NJ * H)
    ).astype(f16)
    bgH = np.ascontiguousarray((b_ih + b_hh).reshape(16, P).T).astype(np.float32)
    bcH = np.ascontiguousarray(b_c.reshape(NJ, P).T).astype(np.float32)
    bhcH = np.ascontiguousarray(b_hc.reshape(NJ, P).T).astype(np.float32)
    return WgH, WcH, WhcH, bgH, bcH, bhcH


def _pack_acts(a, dtype):
    # [BL, H] -> [p, kc*BL + b] with a[b, kc*128+p]
    return np.ascontiguousarray(
        a.T.reshape(KC, P, BL).transpose(1, 0, 2).reshape(P, KC * BL)
    ).astype(dtype)


def kernel(input, hx, W_ih, b_ih, W_hh, b_hh, W_c, b_c, W_hc, b_hc):
    input = np.asarray(input, np.float32)
    hx = np.asarray(hx, np.float32)
    if "nc" not in _CACHE:
        _CACHE["nc"] = _build_program()
    nc = _CACHE["nc"]

    WgH, WcH, WhcH, bgH, bcH, bhcH = _pack_weights(
        np.asarray(W_ih, np.float32), np.asarray(b_ih, np.float32),
        np.asarray(W_hh, np.float32), np.asarray(b_hh, np.float32),
        np.asarray(W_c, np.float32), np.asarray(b_c, np.float32),
        np.asarray(W_hc, np.float32), np.asarray(b_hc, np.float32),
    )

    in_maps = []
    for i in range(N_CORES):
        xs = input[i * BL : (i + 1) * BL]
        hs = hx[i * BL : (i + 1) * BL]
        in_maps.append(
            {
                "xT": _pack_acts(xs, np.float16),
                "hT": _pack_acts(hs, np.float16),
                "hT32": _pack_acts(hs, np.float32),
                "Wg": WgH,
                "Wc": WcH,
                "Whc": WhcH,
                "bg": bgH,
                "bc": bcH,
                "bhc": bhcH,
            }
        )

    res = run_bass_kernel_spmd(nc, in_maps, core_ids=list(range(N_CORES)))
    out = np.empty((B, H), np.float32)
    for i, r in enumerate(res.results):
        o = r["outT"].reshape(P, NJ, BL).transpose(2, 1, 0).reshape(BL, H)
        out[i * BL : (i + 1) * BL] = o
    return out


# revision 15
# speedup vs baseline: 1.0658x; 1.0180x over previous
"""GRU-cell-variant kernel for Trainium2, data-parallel over batch on 8 cores.

Reference (per batch row b, hidden size H=1024):
    gates = sigmoid(x @ W_ih + b_ih + h @ W_hh + b_hh)   # [B, 2H]
    z, r  = gates[:, :H], gates[:, H:]
    cand  = tanh(x @ W_c + b_c + r * (h @ W_hc + b_hc))
    out   = (1 - z) * h + z * cand

Design:
  - 8-way batch shard (1024 rows/core), weights replicated. No collectives.
  - Everything on-chip is computed TRANSPOSED: out.T[o, b]. That way weight
    tiles [k, o] load naturally as the stationary operand, host-pre-transposed
    x.T / h.T serve as the moving operand, and all biases are per-partition
    (free bias-add on the ACT engine).
  - Matmuls in fp16 (1 cycle/row on the PE) with fp32 PSUM accumulation;
    elementwise math and h-residual in fp32.
  - Host packs weights/activations into the exact SBUF layouts so every DMA
    is a dense 2D copy with >=2KB per-partition lines.
"""

import numpy as np

import concourse.bass as bass
import concourse.mybir as mybir
import concourse.tile as tile
from concourse import bacc
from concourse.bass_utils import run_bass_kernel_spmd

N_CORES = 8
B = 8192
H = 1024
BL = B // N_CORES  # batch rows per core
P = 128
KC = H // P  # 8 contraction chunks of 128 per 1024-wide operand
NJ = H // P  # 8 hidden-dim tiles
NB = BL // 512  # 2 moving halves of 512 batch columns

F16 = mybir.dt.float16
F32 = mybir.dt.float32
AF = mybir.ActivationFunctionType
ALU = mybir.AluOpType

_CACHE = {}


def _build_program():
    nc = bacc.Bacc(
        "TRN2",
        target_bir_lowering=False,
        debug=False,
        enable_asserts=False,
        num_devices=N_CORES,
    )

    # DRAM inputs, already packed on the host into SBUF-friendly layouts.
    # xT/hT:  [p, kc*BL + b]        = x[b, kc*128 + p]           (fp16)
    # hT32:   same layout, fp32 (residual path)
    # Wg:     [p, t*2048 + kc*128 + jj] = Wg_full[kc*128+p, t*128+jj]  (fp16)
    #          t in [0,16): gate output tile; kc in [0,16): contraction over [x;h]
    # Wc/Whc: [p, j*1024 + kc*128 + jj] = W[kc*128+p, j*128+jj]  (fp16)
    # bg:     [p, t] = (b_ih+b_hh)[t*128+p]; bc/bhc analogous.
    xT = nc.dram_tensor("xT", [P, KC * BL], F16, kind="ExternalInput").ap()
    hT = nc.dram_tensor("hT", [P, KC * BL], F16, kind="ExternalInput").ap()
    hT32 = nc.dram_tensor("hT32", [P, NJ * BL], F32, kind="ExternalInput").ap()
    Wg = nc.dram_tensor("Wg", [P, 16 * 2048], F16, kind="ExternalInput").ap()
    Wc = nc.dram_tensor("Wc", [P, NJ * H], F16, kind="ExternalInput").ap()
    Whc = nc.dram_tensor("Whc", [P, NJ * H], F16, kind="ExternalInput").ap()
    bg = nc.dram_tensor("bg", [P, 16], F32, kind="ExternalInput").ap()
    bc = nc.dram_tensor("bc", [P, NJ], F32, kind="ExternalInput").ap()
    bhc = nc.dram_tensor("bhc", [P, NJ], F32, kind="ExternalInput").ap()
    outT = nc.dram_tensor("outT", [P, NJ * BL], F32, kind="ExternalOutput").ap()

    with tile.TileContext(nc) as tc:
        with (
            tc.tile_pool(name="const", bufs=1) as cpool,
            tc.tile_pool(name="wg", bufs=4) as wgpool,
            tc.tile_pool(name="wsm", bufs=4) as wsmpool,
            tc.tile_pool(name="psum", bufs=8, space="PSUM") as ppool,
            tc.tile_pool(name="gates", bufs=6) as gpool,
            tc.tile_pool(name="work", bufs=10) as wpool,
        ):
            # Constants are DMA'd on the ACT ring below, interleaved with the
            # j=0 weight chunks (each DMA issue costs ~600ns of sequencer
            # time; the two HWDGE rings issue in parallel).
            bg_sb = cpool.tile([P, 16], F32, tag="bg")
            bc_sb = cpool.tile([P, NJ], F32, tag="bc")
            bhc_sb = cpool.tile([P, NJ], F32, tag="bhc")

            # Resident activations, loaded in per-kc chunks so the first
            # matmuls only wait on the first 128KB-256KB of traffic instead
            # of the full 8MB input preamble. hT32 (residual path, fp32) is
            # streamed per-j inside the loop — it isn't needed until the
            # first elementwise stage.
            xT_sb = cpool.tile([P, KC * BL], F16, tag="xT")
            hT_sb = cpool.tile([P, KC * BL], F16, tag="hT")
            hT32_sb = cpool.tile([P, NJ * BL], F32, tag="hT32")

            def gate_matmuls(psum, w_sb, b0):
                # accumulate over [x;h]: kc<8 reads xT, kc>=8 reads hT
                for kc in range(2 * KC):
                    src = xT_sb if kc < KC else hT_sb
                    off = (kc % KC) * BL + b0
                    nc.tensor.matmul(
                        psum[:],
                        lhsT=w_sb[:, kc * P : (kc + 1) * P],
                        rhs=src[:, off : off + 512],
                        start=(kc == 0),
                        stop=(kc == 2 * KC - 1),
                    )

            def cand_matmuls(psum, w_sb, src, b0):
                for kc in range(KC):
                    off = kc * BL + b0
                    nc.tensor.matmul(
                        psum[:],
                        lhsT=w_sb[:, kc * P : (kc + 1) * P],
                        rhs=src[:, off : off + 512],
                        start=(kc == 0),
                        stop=(kc == KC - 1),
                    )

            def load_wg(dst, t, chunks=1):
                cw = 2048 // chunks
                for c in range(chunks):
                    nc.sync.dma_start(
                        dst[:, c * cw : (c + 1) * cw],
                        Wg[:, t * 2048 + c * cw : t * 2048 + (c + 1) * cw],
                    )

            # 3D views for merged strided chunk loads: [p, kc, b]
            xs3 = xT_sb[:].rearrange("p (kc b) -> p kc b", kc=KC)
            xd3 = xT.rearrange("p (kc b) -> p kc b", kc=KC)
            hs3 = hT_sb[:].rearrange("p (kc b) -> p kc b", kc=KC)
            hd3 = hT.rearrange("p (kc b) -> p kc b", kc=KC)

            def load_act(dst3, src3, kc0, kc1, b0, bw):
                nc.sync.dma_start(
                    dst3[:, kc0:kc1, b0 : b0 + bw], src3[:, kc0:kc1, b0 : b0 + bw]
                )

            for j in range(NJ):
                wz = wgpool.tile([P, 2048], F16, tag="wg")
                wr = wgpool.tile([P, 2048], F16, tag="wg")
                whc_w = wsmpool.tile([P, H], F16, tag="wsm")
                wc_w = wsmpool.tile([P, H], F16, tag="wsm")
                if j == 0:
                    # Cold-start feed across BOTH HWDGE rings so the issue
                    # streams run in parallel: activations on the sync ring,
                    # weights + constants on the ACT ring. The r-gate weights
                    # ride along early because the r matmuls reuse the same
                    # activation bytes (double PE work per DMA'd byte).
                    def wgc(dst, t, c):  # [128, 512] chunk c of gate col t
                        nc.scalar.dma_start(
                            dst[:, c * 512 : (c + 1) * 512],
                            Wg[:, t * 2048 + c * 512 : t * 2048 + (c + 1) * 512],
                        )

                    # sync ring: 6 issues, 4MB
                    load_act(xs3, xd3, 0, 4, 0, 512)   # x kc0-3 b0
                    load_act(xs3, xd3, 4, 8, 0, 512)   # x kc4-7 b0
                    load_act(hs3, hd3, 0, 4, 0, 512)   # h kc0-3 b0
                    load_act(hs3, hd3, 4, 8, 0, 512)   # h kc4-7 b0
                    load_act(xs3, xd3, 0, 8, 512, 512)  # x b1, one issue
                    load_act(hs3, hd3, 0, 8, 512, 512)  # h b1, one issue
                    # ACT ring: weights in arrival-matched order + constants
                    wgc(wz, 0, 0)
                    wgc(wr, NJ, 0)
                    nc.scalar.dma_start(bg_sb[:], bg[:])
                    wgc(wz, 0, 1)
                    wgc(wr, NJ, 1)
                    nc.scalar.dma_start(bc_sb[:], bc[:])
                    nc.scalar.dma_start(bhc_sb[:], bhc[:])
                    wgc(wz, 0, 2)
                    wgc(wr, NJ, 2)
                    wgc(wz, 0, 3)
                    wgc(wr, NJ, 3)
                    nc.scalar.dma_start(whc_w[:], Whc[:, 0:H])
                    nc.scalar.dma_start(wc_w[:], Wc[:, 0:H])
                else:
                    load_wg(wz, j)
                    load_wg(wr, NJ + j)
                    nc.sync.dma_start(whc_w[:], Whc[:, j * H : (j + 1) * H])
                    nc.sync.dma_start(wc_w[:], Wc[:, j * H : (j + 1) * H])
                # residual-path h (fp32) rides the second HWDGE ring (ACT)
                nc.scalar.dma_start(
                    hT32_sb[:, j * BL : (j + 1) * BL], hT32[:, j * BL : (j + 1) * BL]
                )

                for b in range(NB):
                    b0 = b * 512
                    hoff = j * BL + b0  # slice of hidden tile j in [p, j*BL+b] layout

                    pz = ppool.tile([P, 512], F32, tag="ps")
                    if j == 0 and b == 0:
                        # cold start: interleave z/r accumulation in 4-kc
                        # blocks matching the DMA bundle arrival order (PE
                        # executes its stream in order)
                        pr = ppool.tile([P, 512], F32, tag="ps")
                        for c in range(4):
                            for grp, w_sb in ((pz, wz), (pr, wr)):
                                for kc in range(4 * c, 4 * c + 4):
                                    src = xT_sb if kc < KC else hT_sb
                                    off = (kc % KC) * BL + b0
                                    nc.tensor.matmul(
                                        grp[:],
                                        lhsT=w_sb[:, kc * P : (kc + 1) * P],
                                        rhs=src[:, off : off + 512],
                                        start=(kc == 0),
                                        stop=(kc == 2 * KC - 1),
                                    )
                    else:
                        gate_matmuls(pz, wz, b0)
                        pr = None
                    z_sb = gpool.tile([P, 512], F32, tag="g")
                    nc.scalar.activation(z_sb[:], pz[:], AF.Sigmoid, bias=bg_sb[:, j : j + 1])
                    # zh = (z - 1) * h, computed off the critical path so the
                    # post-tanh chain is only mul + subtract
                    zh = wpool.tile([P, 512], F32, tag="w")
                    nc.vector.scalar_tensor_tensor(
                        zh[:], z_sb[:], 1.0, hT32_sb[:, hoff : hoff + 512],
                        ALU.subtract, ALU.mult,
                    )

                    if pr is None:
                        pr = ppool.tile([P, 512], F32, tag="ps")
                        gate_matmuls(pr, wr, b0)
                    r_sb = gpool.tile([P, 512], F32, tag="g")
                    nc.scalar.activation(
                        r_sb[:], pr[:], AF.Sigmoid, bias=bg_sb[:, NJ + j : NJ + j + 1]
                    )

                    ph = ppool.tile([P, 512], F32, tag="ps")
                    cand_matmuls(ph, whc_w, hT_sb, b0)
                    px = ppool.tile([P, 512], F32, tag="ps")
                    cand_matmuls(px, wc_w, xT_sb, b0)

                    # rh = (hc + b_hc) * r   (one DVE op)
                    rh = wpool.tile([P, 512], F32, tag="w")
                    nc.vector.scalar_tensor_tensor(
                        rh[:], ph[:], bhc_sb[:, j : j + 1], r_sb[:], ALU.add, ALU.mult
                    )
                    s = wpool.tile([P, 512], F32, tag="w")
                    nc.vector.tensor_add(s[:], px[:], rh[:])
                    cand = wpool.tile([P, 512], F32, tag="w")
                    nc.scalar.activation(cand[:], s[:], AF.Tanh, bias=bc_sb[:, j : j + 1])

                    # out = z*cand - (z-1)*h
                    m = wpool.tile([P, 512], F32, tag="w")
                    nc.vector.tensor_mul(m[:], z_sb[:], cand[:])
                    o_sb = wpool.tile([P, 512], F32, tag="w")
                    nc.vector.tensor_sub(o_sb[:], m[:], zh[:])
                    nc.scalar.dma_start(outT[:, hoff : hoff + 512], o_sb[:])

    nc.compile()
    return nc


def _pack_weights(W_ih, b_ih, W_hh, b_hh, W_c, b_c, W_hc, b_hc):
    f16 = np.float16
    Wg_full = np.concatenate([W_ih, W_hh], axis=0)  # [2H, 2H] = [k, o]
    WgH = np.ascontiguousarray(
        Wg_full.reshape(16, P, 16, P).transpose(1, 2, 0, 3).reshape(P, 16 * 2048)
    ).astype(f16)
    WcH = np.ascontiguousarray(
        W_c.reshape(KC, P, NJ, P).transpose(1, 2, 0, 3).reshape(P, NJ * H)
    ).astype(f16)
    WhcH = np.ascontiguousarray(
        W_hc.reshape(KC, P, NJ, P).transpose(1, 2, 0, 3).reshape(P, NJ * H)
    ).astype(f16)
    bgH = np.ascontiguousarray((b_ih + b_hh).reshape(16, P).T).astype(np.float32)
    bcH = np.ascontiguousarray(b_c.reshape(NJ, P).T).astype(np.float32)
    bhcH = np.ascontiguousarray(b_hc.reshape(NJ, P).T).astype(np.float32)
    return WgH, WcH, WhcH, bgH, bcH, bhcH


def _pack_acts(a, dtype):
    # [BL, H] -> [p, kc*BL + b] with a[b, kc*128+p]
    return np.ascontiguousarray(
        a.T.reshape(KC, P, BL).transpose(1, 0, 2).reshape(P, KC * BL)
    ).astype(dtype)


def kernel(input, hx, W_ih, b_ih, W_hh, b_hh, W_c, b_c, W_hc, b_hc):
    input = np.asarray(input, np.float32)
    hx = np.asarray(hx, np.float32)
    if "nc" not in _CACHE:
        _CACHE["nc"] = _build_program()
    nc = _CACHE["nc"]

    WgH, WcH, WhcH, bgH, bcH, bhcH = _pack_weights(
        np.asarray(W_ih, np.float32), np.asarray(b_ih, np.float32),
        np.asarray(W_hh, np.float32), np.asarray(b_hh, np.float32),
        np.asarray(W_c, np.float32), np.asarray(b_c, np.float32),
        np.asarray(W_hc, np.float32), np.asarray(b_hc, np.float32),
    )

    in_maps = []
    for i in range(N_CORES):
        xs = input[i * BL : (i + 1) * BL]
        hs = hx[i * BL : (i + 1) * BL]
        in_maps.append(
            {
                "xT": _pack_acts(xs, np.float16),
                "hT": _pack_acts(hs, np.float16),
                "hT32": _pack_acts(hs, np.float32),
                "Wg": WgH,
                "Wc": WcH,
                "Whc": WhcH,
                "bg": bgH,
                "bc": bcH,
                "bhc": bhcH,
            }
        )

    res = run_bass_kernel_spmd(nc, in_maps, core_ids=list(range(N_CORES)))
    out = np.empty((B, H), np.float32)
    for i, r in enumerate(res.results):
        o = r["outT"].reshape(P, NJ, BL).transpose(2, 1, 0).reshape(BL, H)
        out[i * BL : (i + 1) * BL] = o
    return out
